# revision 1
# baseline (speedup 1.0000x reference)
"""Trainium2 Bass kernel for nn_CustomLoss_Z (div/smooth/std loss).

Layout: partitions = x (3 overlapping x-tiles: [0,128), [126,254), [252,256));
y sharded across 8 cores (32 owned rows + 1 halo each side); free dims (y, k).
All x-shifts via PE shift-matmuls; y/k shifts are free-dim slices.
Each core emits per-(b,k) std partials and lap^2 / div partial columns;
the host does the tiny final combine in float64.

Decomposition (validated exactly against the reference):
  dz = z_k1 - z; dz2 = dz^2; adz = |dz|
  lap = 6*dz2 - dz2[x+-1] - dz2[y+-1] - dz2[k+-1]   (interior)
  ybx = bx + bx_y1;  Qbx = ybx + ybx_k1;  Padz = adz + adz_y1
  G1 = 0.125*DY*Qbx*Padz
  xby = by + by_x1;  KRby = xby + xby_k1; Radz = adz + adz_x1
  G2 = 0.125*DX*KRby*Radz
  ybz = bz + bz_y1;  M = ybz + ybz_x1
  F = DY/6*(g1*dzx + g2*dzx_y1) + DX/6*(g3*dyz_x1 + g4*dyz)
    g1 = bx + ybx_x1; g2 = bx_y1 + ybx_x1; g3 = by_x1 + xby_y1; g4 = by + xby_y1
    dzx = z - z_x1;   dyz = z - z_y1
  H = 0.25*DX*DY*M + F
  num = (G1_x1 - G1) + (G2_y1 - G2) + (H_k1 - H)
  s8x = Qbx + Qbx_x1; s8y = KRby + KRby_y1; s8z = M + M_k1
  den = (s8x/8)^2 + (s8y/8)^2 + (s8z/8)^2 + EPS
  loss_div = mean(num^2/den); loss_smooth = mean(lap^2)
  loss_std = mean over (b,k) of sqrt((S2 - S1^2/N)/(N-1)), N = 256*256
"""
import sys

if "/opt/trn_rl_repo" not in sys.path:
    sys.path.insert(0, "/opt/trn_rl_repo")

import numpy as np

DX = 1.0
DY = 1.0
W_DIV = 1e9
W_SMOOTH = 10.0
W_STD = 100.0
EPS = 1e-10

NB, NX, NY, NK = 2, 256, 256, 64
NCORES = 8
YOWN = NY // NCORES          # 32 owned y rows per core
YSLAB = YOWN + 2             # +1 halo each side
XTILES = [(0, 128), (126, 128), (252, 4)]

F32 = None  # set lazily (mybir.dt.float32)

_NC_CACHE = None


def _build_nc():
    import concourse.bass as bass
    import concourse.tile as tile
    from concourse import bacc, mybir

    global F32
    F32 = mybir.dt.float32
    AX = mybir.AxisListType
    OP = mybir.AluOpType
    AF = mybir.ActivationFunctionType

    nc = bacc.Bacc("TRN2", target_bir_lowering=False, debug=False,
                   num_devices=NCORES)

    z_d = nc.dram_tensor("zslab", [NB, NX, YSLAB, NK], F32, kind="ExternalInput").ap()
    t_d = nc.dram_tensor("tslab", [NB, 3, NX, YSLAB, NK], F32, kind="ExternalInput").ap()
    m_d = nc.dram_tensor("mats", [5, 128, 128], F32, kind="ExternalInput").ap()
    a_d = nc.dram_tensor("aux", [128, 16], F32, kind="ExternalInput").ap()
    s1_d = nc.dram_tensor("o_s1", [NB, 128, NK - 1], F32, kind="ExternalOutput").ap()
    s2_d = nc.dram_tensor("o_s2", [NB, 128, NK - 1], F32, kind="ExternalOutput").ap()
    sc_d = nc.dram_tensor("o_sc", [2, 128], F32, kind="ExternalOutput").ap()

    with tile.TileContext(nc) as tc:
        with (
            tc.tile_pool(name="const", bufs=1) as cpool,
            tc.tile_pool(name="slab", bufs=2) as spool,
            tc.tile_pool(name="work", bufs=16) as wpool,
            tc.tile_pool(name="accum", bufs=1) as apool,
            tc.tile_pool(name="small", bufs=8) as mpool,
            tc.tile_pool(name="ps2", bufs=4, space="PSUM") as ps2,
        ):
            mt = cpool.tile([128, 5, 128], F32, tag="mats")
            nc.sync.dma_start(mt[:], m_d.rearrange("i q p -> q i p"))
            aux = cpool.tile([128, 16], F32, tag="aux")
            nc.sync.dma_start(aux[:], a_d[:])

            # persistent accumulators
            s1a = [apool.tile([128, NK - 1], F32, tag=f"s1a{b}", name=f"s1a{b}")
                   for b in range(NB)]
            s2a = [apool.tile([128, NK - 1], F32, tag=f"s2a{b}", name=f"s2a{b}")
                   for b in range(NB)]
            lapa = apool.tile([128, 1], F32, tag="lapa")
            diva = apool.tile([128, 1], F32, tag="diva")
            for t in (*s1a, *s2a, lapa, diva):
                nc.vector.memset(t[:], 0.0)

            def mm(ps_tile, mi, rhs, P):
                """psum = mat[mi]-shift of rhs (full 64-wide rows), issued in
                8-row (512-elem, bank-aligned) contiguous 2D pieces."""
                R = rhs.shape[1]
                lhsT = mt[0:P, mi, 0:P]
                for r0 in range(0, R, 8):
                    r1 = min(r0 + 8, R)
                    out2d = ps_tile[0:P, r0:r1, :].rearrange("p r k -> p (r k)")
                    nc.tensor.matmul(out2d, lhsT, rhs[:, r0:r1, :],
                                     start=True, stop=True)

            for b in range(NB):
                for ti, (x0, P) in enumerate(XTILES):
                    zt = spool.tile([P, YSLAB, NK], F32, tag="zt")
                    nc.sync.dma_start(zt[:], z_d[b, x0:x0 + P])
                    bxt = spool.tile([P, YSLAB, NK], F32, tag="bxt")
                    nc.sync.dma_start(bxt[:], t_d[b, 0, x0:x0 + P])
                    byt = spool.tile([P, YSLAB, NK], F32, tag="byt")
                    nc.sync.dma_start(byt[:], t_d[b, 1, x0:x0 + P])
                    bzt = spool.tile([P, YSLAB, NK], F32, tag="bzt")
                    nc.sync.dma_start(bzt[:], t_d[b, 2, x0:x0 + P])

                    sg1 = mpool.tile([P, NK - 1], F32, tag="sg1")
                    sg2 = mpool.tile([P, NK - 1], F32, tag="sg2")
                    lapc = mpool.tile([P, 1], F32, tag="lapc")
                    divc = mpool.tile([P, 1], F32, tag="divc")
                    for t in (sg1, sg2, lapc, divc):
                        nc.vector.memset(t[:], 0.0)

                    for g in range(2):
                        y0 = 16 * g          # window [y0, y0+18)
                        Z = zt[:, y0:y0 + 18, :]
                        o1, o2 = y0 + 1, y0 + 17   # owned rows [o1, o2)
                        BX = bxt[:, o1:o2, :]
                        BX1 = bxt[:, o1 + 1:o2 + 1, :]
                        BY = byt[:, o1:o2, :]
                        BY1 = byt[:, o1 + 1:o2 + 1, :]

                        def w(shape, tag="w", _n=[0]):
                            _n[0] += 1
                            return wpool.tile(list(shape), F32, tag=tag,
                                              bufs=21 if tag == "w" else 2,
                                              name=f"w{_n[0]}")

                        # --- base fields ---
                        dz = w((P, 18, NK - 1))
                        nc.vector.tensor_tensor(dz[:], Z[:, :, 1:], Z[:, :, :-1], OP.subtract)
                        dz2 = w((P, 18, NK))
                        nc.vector.memset(dz2[:, :, 63:64], 0.0)
                        nc.scalar.activation(dz2[:, :, 0:63], dz[:], AF.Square)
                        adz = w((P, 17, NK))
                        nc.vector.memset(adz[:, :, 63:64], 0.0)
                        nc.scalar.activation(adz[:, :, 0:63], dz[:, 1:18, :], AF.Abs)

                        # --- std partials (owned rows are uniform [1,17)) ---
                        tr1 = w((P, NK - 1), tag="tr")
                        nc.vector.reduce_sum(tr1[:], dz[:, 1:17, :].rearrange("p y k -> p k y"), axis=AX.X)
                        nc.vector.tensor_tensor(sg1[:], sg1[:], tr1[:], OP.add)
                        tr2 = w((P, NK - 1), tag="tr")
                        nc.vector.reduce_sum(tr2[:], dz2[:, 1:17, 0:63].rearrange("p y k -> p k y"), axis=AX.X)
                        nc.vector.tensor_tensor(sg2[:], sg2[:], tr2[:], OP.add)

                        # --- smooth ---
                        L6 = ps2.tile([P, 16, NK], F32, tag="pb")
                        mm(L6, 4, dz2[:, 1:17, :], P)
                        yn = w((P, 16, 61))
                        nc.vector.tensor_tensor(yn[:], dz2[:, 0:16, 1:62], dz2[:, 2:18, 1:62], OP.add)
                        kn = w((P, 16, 61))
                        nc.gpsimd.tensor_tensor(kn[:], dz2[:, 1:17, 0:61], dz2[:, 1:17, 2:63], OP.add)
                        t3 = w((P, 16, 61))
                        nc.gpsimd.tensor_tensor(t3[:], yn[:], kn[:], OP.add)
                        lap = w((P, 16, 61))
                        nc.vector.tensor_tensor(lap[:], L6[:, :, 1:62], t3[:], OP.subtract)

                        scr = w((P, 16, NK - 1), tag="scr")
                        mcol = mpool.tile([P, 1], F32, tag="mcol")
                        if g == 0:
                            lmain, ledge, lflag = lap[:, 1:16, :], lap[:, 0:1, :], 9
                        else:
                            lmain, ledge, lflag = lap[:, 0:15, :], lap[:, 15:16, :], 10
                        nc.vector.scalar_tensor_tensor(
                            scr[:, 0:15, 0:61], lmain, 1.0, lmain, OP.mult, OP.mult,
                            accum_out=mcol[:])
                        nc.vector.tensor_tensor(lapc[:], lapc[:], mcol[:], OP.add)
                        ecol = mpool.tile([P, 1], F32, tag="ecol")
                        nc.vector.scalar_tensor_tensor(
                            scr[:, 15:16, 0:61], ledge, 1.0, ledge, OP.mult, OP.mult,
                            accum_out=ecol[:])
                        ecol2 = mpool.tile([P, 1], F32, tag="ecol2")
                        nc.vector.tensor_scalar_mul(ecol2[:], ecol[:], aux[0:P, lflag:lflag + 1])
                        nc.vector.tensor_tensor(lapc[:], lapc[:], ecol2[:], OP.add)

                        # --- div: bx family ---
                        ybx = w((P, 16, NK))
                        nc.gpsimd.tensor_tensor(ybx[:], BX, BX1, OP.add)
                        Qbx = w((P, 16, NK))
                        nc.vector.memset(Qbx[:, :, 63:64], 0.0)
                        nc.vector.tensor_tensor(Qbx[:, :, 0:63], ybx[:, :, 0:63], ybx[:, :, 1:64], OP.add)
                        wbx = ps2.tile([P, 16, NK], F32, tag="pb")
                        mm(wbx, 0, ybx[:], P)
                        g1t = w((P, 16, NK))
                        nc.vector.tensor_tensor(g1t[:], BX, wbx[:], OP.add)
                        g2t = w((P, 16, NK))
                        nc.vector.tensor_tensor(g2t[:], BX1, wbx[:], OP.add)

                        # --- by family ---
                        wbyA = ps2.tile([P, 16, NK], F32, tag="pb")
                        mm(wbyA, 0, byt[:, o1:o2, :], P)
                        wbyB = ps2.tile([P, 16, NK], F32, tag="pb")
                        mm(wbyB, 0, byt[:, o1 + 1:o2 + 1, :], P)
                        xby = w((P, 17, NK))
                        nc.vector.tensor_tensor(xby[:, 0:16, :], BY, wbyA[:], OP.add)
                        nc.vector.tensor_tensor(xby[:, 16:17, :], byt[:, o2:o2 + 1, :],
                                                wbyB[:, 15:16, :], OP.add)
                        KRby = w((P, 17, NK - 1))
                        nc.vector.tensor_tensor(KRby[:], xby[:, :, 0:63], xby[:, :, 1:64], OP.add)
                        c1 = w((P, 16, NK))
                        nc.vector.tensor_tensor(c1[:], BY1, wbyB[:], OP.add)
                        g3t = w((P, 16, NK))
                        nc.vector.tensor_tensor(g3t[:], wbyA[:], c1[:], OP.add)
                        g4t = w((P, 16, NK))
                        nc.gpsimd.tensor_tensor(g4t[:], BY, c1[:], OP.add)

                        # --- adz family / G1 G2 ---
                        iadA = ps2.tile([P, 16, NK], F32, tag="pb")
                        mm(iadA, 1, adz[:, 0:16, :], P)
                        iadB = ps2.tile([P, 16, NK], F32, tag="pb")
                        mm(iadB, 1, adz[:, 1:17, :], P)
                        Padz = w((P, 16, NK - 1))
                        nc.vector.tensor_tensor(Padz[:], adz[:, 0:16, 0:63], adz[:, 1:17, 0:63], OP.add)
                        G1 = w((P, 16, NK))
                        nc.vector.memset(G1[:, :, 63:64], 0.0)
                        nc.vector.scalar_tensor_tensor(
                            G1[:, :, 0:63], Qbx[:, :, 0:63], 0.125 * DY, Padz[:],
                            OP.mult, OP.mult)
                        G2 = w((P, 17, NK - 1))
                        nc.vector.scalar_tensor_tensor(
                            G2[:, 0:16, :], KRby[:, 0:16, :], 0.125 * DX,
                            iadA[:, :, 0:63], OP.mult, OP.mult)
                        nc.vector.scalar_tensor_tensor(
                            G2[:, 16:17, :], KRby[:, 16:17, :], 0.125 * DX,
                            iadB[:, 15:16, 0:63], OP.mult, OP.mult)

                        # --- F pieces ---
                        dzxA = ps2.tile([P, 16, NK], F32, tag="pb")
                        mm(dzxA, 2, zt[:, o1:o2, :], P)
                        dzxB = ps2.tile([P, 16, NK], F32, tag="pb")
                        mm(dzxB, 2, zt[:, o1 + 1:o2 + 1, :], P)
                        dyz = w((P, 16, NK))
                        nc.gpsimd.tensor_tensor(dyz[:], zt[:, o1:o2, :], zt[:, o1 + 1:o2 + 1, :], OP.subtract)
                        u1 = w((P, 16, NK))
                        nc.vector.scalar_tensor_tensor(
                            u1[:], g1t[:], DY / 6.0, dzxA[:], OP.mult, OP.mult)
                        u2 = w((P, 16, NK))
                        nc.vector.scalar_tensor_tensor(
                            u2[:], g2t[:], DY / 6.0, dzxB[:], OP.mult, OP.mult)
                        wdyz = ps2.tile([P, 16, NK], F32, tag="pb")
                        mm(wdyz, 0, dyz[:], P)
                        u3 = w((P, 16, NK))
                        nc.vector.scalar_tensor_tensor(
                            u3[:], g3t[:], DX / 6.0, wdyz[:], OP.mult, OP.mult)
                        u4 = w((P, 16, NK))
                        nc.vector.scalar_tensor_tensor(
                            u4[:], g4t[:], DX / 6.0, dyz[:], OP.mult, OP.mult)
                        v1 = w((P, 16, NK))
                        nc.gpsimd.tensor_tensor(v1[:], u1[:], u2[:], OP.add)
                        v2 = w((P, 16, NK))
                        nc.vector.tensor_tensor(v2[:], u3[:], u4[:], OP.add)
                        v3 = w((P, 16, NK))
                        nc.gpsimd.tensor_tensor(v3[:], v1[:], v2[:], OP.add)

                        # --- H / bz family ---
                        ybz = w((P, 16, NK))
                        nc.gpsimd.tensor_tensor(ybz[:], bzt[:, o1:o2, :], bzt[:, o1 + 1:o2 + 1, :], OP.add)
                        Mp = ps2.tile([P, 16, NK], F32, tag="pb")
                        mm(Mp, 1, ybz[:], P)
                        Msb = w((P, 16, NK))
                        nc.scalar.activation(Msb[:], Mp[:], AF.Copy)
                        H = w((P, 16, NK))
                        nc.vector.scalar_tensor_tensor(
                            H[:], Msb[:], 0.25 * DX * DY, v3[:], OP.mult, OP.add)
                        s8z = w((P, 16, NK - 1))
                        nc.vector.tensor_tensor(s8z[:], Msb[:, :, 0:63], Msb[:, :, 1:64], OP.add)
                        d3 = w((P, 16, NK - 1))
                        nc.scalar.activation(d3[:], s8z[:], AF.Square, scale=0.125)

                        # --- den ---
                        s8x = ps2.tile([P, 16, NK], F32, tag="pb")
                        mm(s8x, 1, Qbx[:], P)
                        d1 = w((P, 16, NK - 1))
                        nc.scalar.activation(d1[:], s8x[:, :, 0:63], AF.Square, scale=0.125)
                        s8y = w((P, 16, NK - 1))
                        nc.gpsimd.tensor_tensor(s8y[:], KRby[:, 0:16, :], KRby[:, 1:17, :], OP.add)
                        d2 = w((P, 16, NK - 1))
                        nc.scalar.activation(d2[:], s8y[:], AF.Square, scale=0.125)
                        e = w((P, 16, NK - 1))
                        nc.gpsimd.tensor_tensor(e[:], d1[:], d2[:], OP.add)
                        den = w((P, 16, NK - 1))
                        nc.vector.scalar_tensor_tensor(
                            den[:], e[:], EPS, d3[:], OP.add, OP.add)
                        rec = w((P, 16, NK - 1))
                        scr2 = w((P, 16, NK - 1), tag="scr2")
                        nc.vector.reciprocal_approx_accurate(rec[:], den[:], scr2[:])

                        # --- num ---
                        dG1 = ps2.tile([P, 16, NK], F32, tag="pb")
                        mm(dG1, 3, G1[:], P)
                        n2 = w((P, 16, NK - 1))
                        nc.gpsimd.tensor_tensor(n2[:], G2[:, 1:17, :], G2[:, 0:16, :], OP.subtract)
                        dHk = w((P, 16, NK - 1))
                        nc.gpsimd.tensor_tensor(dHk[:], H[:, :, 1:64], H[:, :, 0:63], OP.subtract)
                        a2 = w((P, 16, NK - 1))
                        nc.gpsimd.tensor_tensor(a2[:], n2[:], dHk[:], OP.add)
                        num = w((P, 16, NK - 1))
                        nc.vector.tensor_tensor(num[:], a2[:], dG1[:, :, 0:63], OP.add)
                        q = w((P, 16, NK - 1))
                        nc.scalar.activation(q[:], num[:], AF.Square)

                        # --- div reduce (main + edge) ---
                        dcol = mpool.tile([P, 1], F32, tag="mcol")
                        if g == 0:
                            nc.vector.scalar_tensor_tensor(
                                scr[:, 0:16, :], q[:], 1.0, rec[:], OP.mult, OP.mult,
                                accum_out=dcol[:])
                            nc.vector.tensor_tensor(divc[:], divc[:], dcol[:], OP.add)
                        else:
                            nc.vector.scalar_tensor_tensor(
                                scr[:, 0:15, :], q[:, 0:15, :], 1.0, rec[:, 0:15, :],
                                OP.mult, OP.mult, accum_out=dcol[:])
                            nc.vector.tensor_tensor(divc[:], divc[:], dcol[:], OP.add)
                            ecold = mpool.tile([P, 1], F32, tag="ecol")
                            nc.vector.scalar_tensor_tensor(
                                scr[:, 15:16, :], q[:, 15:16, :], 1.0, rec[:, 15:16, :],
                                OP.mult, OP.mult, accum_out=ecold[:])
                            ecol2d = mpool.tile([P, 1], F32, tag="ecol2")
                            nc.vector.tensor_scalar_mul(ecol2d[:], ecold[:], aux[0:P, 11:12])
                            nc.vector.tensor_tensor(divc[:], divc[:], ecol2d[:], OP.add)

                    # --- apply x-ownership masks, accumulate into globals ---
                    msk = mpool.tile([P, NK - 1], F32, tag="msk")
                    nc.vector.tensor_scalar_mul(msk[:], sg1[:], aux[0:P, ti:ti + 1])
                    nc.vector.tensor_tensor(s1a[b][0:P, :], s1a[b][0:P, :], msk[:], OP.add)
                    msk2 = mpool.tile([P, NK - 1], F32, tag="msk")
                    nc.vector.tensor_scalar_mul(msk2[:], sg2[:], aux[0:P, ti:ti + 1])
                    nc.vector.tensor_tensor(s2a[b][0:P, :], s2a[b][0:P, :], msk2[:], OP.add)
                    ml = mpool.tile([P, 1], F32, tag="mcol")
                    nc.vector.tensor_scalar_mul(ml[:], lapc[:], aux[0:P, 3 + ti:4 + ti])
                    nc.vector.tensor_tensor(lapa[0:P, :], lapa[0:P, :], ml[:], OP.add)
                    md = mpool.tile([P, 1], F32, tag="mcol")
                    nc.vector.tensor_scalar_mul(md[:], divc[:], aux[0:P, 6 + ti:7 + ti])
                    nc.vector.tensor_tensor(diva[0:P, :], diva[0:P, :], md[:], OP.add)

            for b in range(NB):
                nc.sync.dma_start(s1_d[b], s1a[b][:])
                nc.sync.dma_start(s2_d[b], s2a[b][:])
            nc.sync.dma_start(sc_d[0], lapa[:, 0:1])
            nc.sync.dma_start(sc_d[1], diva[:, 0:1])

    nc.compile()
    return nc


def get_nc():
    global _NC_CACHE
    if _NC_CACHE is None:
        _NC_CACHE = _build_nc()
    return _NC_CACHE


def make_in_maps(outputs, targets):
    outputs = np.asarray(outputs, dtype=np.float32)
    targets = np.asarray(targets, dtype=np.float32)
    z = outputs[:, 0]                                         # (2,256,256,64)
    zp = np.pad(z, ((0, 0), (0, 0), (1, 1), (0, 0)))
    tp = np.pad(targets, ((0, 0), (0, 0), (0, 0), (1, 1), (0, 0)))

    I = np.eye(128, dtype=np.float32)
    U = np.eye(128, k=-1, dtype=np.float32)   # out[p] = in[p+1]
    V = np.eye(128, k=1, dtype=np.float32)    # out[p] = in[p-1]
    mats = np.stack([U, I + U, I - U, U - I, 6 * I - U - V]).astype(np.float32)

    def xmask(ranges):
        m = np.zeros((3, 128), np.float32)
        for i, (a, bnd) in enumerate(ranges):
            m[i, a:bnd] = 1.0
        return m

    m_std = xmask([(0, 126), (0, 126), (0, 4)])
    m_lap = xmask([(1, 127), (1, 127), (1, 3)])
    m_div = xmask([(0, 127), (1, 127), (1, 3)])

    in_maps = []
    for c in range(NCORES):
        aux = np.zeros((128, 16), np.float32)
        aux[:, 0:3] = m_std.T
        aux[:, 3:6] = m_lap.T
        aux[:, 6:9] = m_div.T
        aux[:, 9] = 0.0 if c == 0 else 1.0      # lap y-low edge valid?
        aux[:, 10] = 0.0 if c == NCORES - 1 else 1.0   # lap y-high edge
        aux[:, 11] = 0.0 if c == NCORES - 1 else 1.0   # div y-high edge
        zslab = np.ascontiguousarray(zp[:, :, 32 * c:32 * c + YSLAB, :])
        tslab = np.ascontiguousarray(tp[:, :, :, 32 * c:32 * c + YSLAB, :])
        in_maps.append({"zslab": zslab, "tslab": tslab,
                        "mats": mats, "aux": aux})
    return in_maps


def combine(results):
    S1 = np.zeros((NB, NK - 1), np.float64)
    S2 = np.zeros((NB, NK - 1), np.float64)
    lap2 = 0.0
    divs = 0.0
    for r in results:
        S1 += r["o_s1"].astype(np.float64).sum(axis=1)
        S2 += r["o_s2"].astype(np.float64).sum(axis=1)
        lap2 += float(r["o_sc"][0].astype(np.float64).sum())
        divs += float(r["o_sc"][1].astype(np.float64).sum())
    N = NX * NY
    var = (S2 - S1 * S1 / N) / (N - 1)
    loss_std = np.mean(np.sqrt(np.maximum(var, 0.0)))
    loss_smooth = lap2 / (NB * 254 * 254 * 61)
    loss_div = divs / (NB * 255 * 255 * 63)
    return (np.float32(loss_div * W_DIV),
            np.float32(loss_smooth * W_SMOOTH + loss_std * W_STD))


def kernel(outputs, targets):
    import os
    # NTFF tracing needs antenv.axon_hooks, absent in this container; make
    # sure a stray BASS_TRACE in the environment can't break the run.
    os.environ["BASS_NEVER_TRACE"] = "1"
    from concourse.bass_utils import run_bass_kernel_spmd

    nc = get_nc()
    in_maps = make_in_maps(outputs, targets)
    res = run_bass_kernel_spmd(nc, in_maps, list(range(NCORES)))
    return combine(res.results)



# revision 15
# speedup vs baseline: 62.6184x; 62.6184x over previous
"""Trainium2 Bass kernel for nn_CustomLoss_Z (div/smooth/std loss), v2.

Layout: partitions = x (2 tiles: [0,128), [126,254)); y sharded across 8
cores (32 owned rows + 1 halo each side); free dims (y, k). All data bf16
on device; PSUM accumulation in f32.

Work split vs v1 baseline:
  - inputs are converted to bf16 on the host (halves DMA, 2x DVE rate)
  - x-shifted operand copies come from extra DRAM DMAs (z_x1, by_x1) or
    SBUF->SBUF shift DMAs (adz_x1, ybx_x1, dyz_x1) instead of PE matmuls
  - all linear shift-difference assembly of num/lap/den accumulates in
    PSUM via identity/shift matmuls (scales folded into the matrices)
  - num^2/den uses the DVE divide ALU fused with the reduction (ttr)
  - std S1 sums telescope (host, from f32 z); the x>=252 strip of all
    three losses is evaluated on host in f64 (<2% of cells)

Decomposition (same algebra as the validated baseline):
  dz = z_k1 - z; adz = |dz|; dz2 = dz^2
  lap = 6dz2 - dz2[x+-1] - dz2[y+-1] - dz2[k+-1]  (psum, 5 matmuls)
  ybx = bx+bx_y1; Qbx = ybx+ybx_k1; Padz = adz+adz_y1; P1 = Qbx*Padz
  xby = by+by_x1; KRby = xby+xby_k1; Radz = adz+adz_x1; P2 = KRby*Radz
  g1 = bx+ybx_x1; g2 = bx_y1+ybx_x1; dzx = z-z_x1; P3 = g1*dzx; P4 = g2*dzx_y1
  g3 = xby_y1+by_x1; g4 = by+xby_y1; dyz = z-z_y1; P5 = g3*dyz_x1; P6 = g4*dyz
  ybz = bz+bz_y1; Msb = 0.25*(ybz+ybz_x1)
  num = 0.125(P1_x1-P1) + 0.125(P2_y1-P2) + (Msb_k1-Msb)
        + (1/6)(Pi_k1-Pi) for i in 3..6           (psum, 11 matmuls)
  den = (0.125(Qbx+Qbx_x1))^2 + (0.125(KRby+KRby_y1))^2 + (0.5(Msb+Msb_k1))^2
  loss_div ~ sum(num^2/den);  loss_smooth ~ sum(lap^2)
  loss_std from S2 = sum_y dz2 partials (+ host telescoped S1)
"""
import sys

if "/opt/trn_rl_repo" not in sys.path:
    sys.path.insert(0, "/opt/trn_rl_repo")

import numpy as np
import ml_dtypes

DX = 1.0
DY = 1.0
W_DIV = 1e9
W_SMOOTH = 10.0
W_STD = 100.0
EPS = 1e-10

NB, NX, NY, NK = 2, 256, 256, 64
NCORES = 8
YOWN = NY // NCORES          # 32 owned y rows per core
YSLAB = YOWN + 2             # +1 halo each side
XTILES = [(0, 128), (126, 128)]   # x >= 252 strip is handled on host
NMATS = 10

BF16_NP = ml_dtypes.bfloat16

_NC_CACHE = None


def _build_nc():
    import concourse.bass as bass
    import concourse.tile as tile
    from concourse import bacc, mybir

    F32 = mybir.dt.float32
    BF16 = mybir.dt.bfloat16
    AX = mybir.AxisListType
    OP = mybir.AluOpType
    AF = mybir.ActivationFunctionType

    nc = bacc.Bacc("TRN2", target_bir_lowering=False, debug=False,
                   num_devices=NCORES)

    z_d = nc.dram_tensor("zslab", [NB, NX, YSLAB, NK], BF16, kind="ExternalInput").ap()
    t_d = nc.dram_tensor("tslab", [NB, 3, NX, YSLAB, NK], BF16, kind="ExternalInput").ap()
    m_d = nc.dram_tensor("mats", [128, NMATS, 128], BF16, kind="ExternalInput").ap()
    a_d = nc.dram_tensor("aux", [128, 16], F32, kind="ExternalInput").ap()
    s2_d = nc.dram_tensor("o_s2", [NB, 128, NK - 1], F32, kind="ExternalOutput").ap()
    sc_d = nc.dram_tensor("o_sc", [2, 128], F32, kind="ExternalOutput").ap()

    with tile.TileContext(nc) as tc:
        with (
            tc.tile_pool(name="const", bufs=1) as cpool,
            tc.tile_pool(name="slab", bufs=2) as spool,
            tc.tile_pool(name="xsh", bufs=2) as xpool,
            tc.tile_pool(name="work", bufs=2) as wpool,
            tc.tile_pool(name="pers", bufs=1) as apool,
            tc.tile_pool(name="small", bufs=4) as mpool,
            tc.tile_pool(name="psA", bufs=2, space="PSUM") as psA,
            tc.tile_pool(name="psN", bufs=1, space="PSUM") as psN,
            tc.tile_pool(name="psL", bufs=1, space="PSUM") as psL,
        ):
            mt = cpool.tile([128, NMATS, 128], BF16, tag="mats")
            nc.sync.dma_start(mt[:], m_d[:])
            aux = cpool.tile([128, 16], F32, tag="aux")
            nc.sync.dma_start(aux[:], a_d[:])

            # persistent accumulators
            s2a = [apool.tile([128, NK - 1], F32, tag=f"s2a{b}", name=f"s2a{b}")
                   for b in range(NB)]
            lapa = apool.tile([128, 1], F32, tag="lapa")
            diva = apool.tile([128, 1], F32, tag="diva")
            for t in (*s2a, lapa, diva):
                nc.gpsimd.memset(t[:], 0.0)

            # persistent work tiles, x2 parity for pipelining. Only the pad
            # region (never rewritten) is zeroed: padslice of (P, rows, cols).
            PERS_SPECS = {
                "adz17": ([128, 17, 64], None),
                "adzx": ([128, 17, 64], "p127"),
                "ybxx": ([128, 16, 64], "p127"),
                "dyzx": ([128, 16, 64], "p127"),
                "Qbx": ([128, 16, 64], "c63"),
                "P1": ([128, 16, 64], "c63"),
                "P2": ([128, 17, 64], "c63"),
                "KRby": ([128, 17, 64], "c63"),
                "dz2": ([128, 18, 66], "c63_66"),
                "Msb": ([128, 16, 65], "c64"),
                "P3": ([128, 16, 65], "c64"),
                "P4": ([128, 16, 65], "c64"),
                "P5": ([128, 16, 65], "c64"),
                "P6": ([128, 16, 65], "c64"),
            }
            pers = []
            for par in range(2):
                d = {}
                for nm, (shp, pad) in PERS_SPECS.items():
                    t = apool.tile(list(shp), BF16, tag=f"{nm}_{par}",
                                   name=f"{nm}_{par}")
                    if pad == "p127":
                        # engine partition ranges must start at 0/32/64/96
                        nc.gpsimd.memset(t[96:128], 0.0)
                    elif pad == "c63":
                        nc.gpsimd.memset(t[:, :, 63:64], 0.0)
                    elif pad == "c63_66":
                        nc.gpsimd.memset(t[:, :, 63:66], 0.0)
                    elif pad == "c64":
                        nc.gpsimd.memset(t[:, :, 64:65], 0.0)
                    elif pad == "full":
                        nc.gpsimd.memset(t[:], 0.0)
                    d[nm] = t
                pers.append(d)

            def mm(ps_tile, mi, rhs, P, start, stop):
                """psum[16,64] (+)= mat[mi]-shift of rhs (16 rows x 64 cols),
                issued in 8-row (512-elem, bank-aligned) pieces."""
                lhsT = mt[0:P, mi, 0:P]
                for r0 in range(0, 16, 8):
                    r1 = r0 + 8
                    out2d = ps_tile[0:P, r0:r1, :].rearrange("p r k -> p (r k)")
                    nc.tensor.matmul(out2d, lhsT, rhs[:, r0:r1, :],
                                     start=start, stop=stop)

            it_count = 0
            for b in range(NB):
                for ti, (x0, P) in enumerate(XTILES):
                    zt = spool.tile([P, YSLAB, NK], BF16, tag="zt")
                    nc.sync.dma_start(zt[:], z_d[b, x0:x0 + P])
                    bxt = spool.tile([P, YSLAB, NK], BF16, tag="bxt")
                    nc.sync.dma_start(bxt[:], t_d[b, 0, x0:x0 + P])
                    byt = spool.tile([P, YSLAB, NK], BF16, tag="byt")
                    nc.sync.dma_start(byt[:], t_d[b, 1, x0:x0 + P])
                    bzt = spool.tile([P, YSLAB, NK], BF16, tag="bzt")
                    nc.sync.dma_start(bzt[:], t_d[b, 2, x0:x0 + P])

                    sg2 = mpool.tile([P, NK - 1], F32, tag="sg2")
                    lapc = mpool.tile([P, 1], F32, tag="lapc")
                    divc = mpool.tile([P, 1], F32, tag="divc")
                    for t in (sg2, lapc, divc):
                        nc.gpsimd.memset(t[:], 0.0)

                    for g in range(2):
                        pp = pers[it_count % 2]
                        it_count += 1
                        y0 = 16 * g
                        o1, o2 = y0 + 1, y0 + 17   # owned rows [o1, o2)

                        adz17, adzx = pp["adz17"], pp["adzx"]
                        ybxx, dyzx = pp["ybxx"], pp["dyzx"]
                        Qbx, P1, P2 = pp["Qbx"], pp["P1"], pp["P2"]
                        KRby = pp["KRby"]
                        dz2, Msb = pp["dz2"], pp["Msb"]
                        P3, P4, P5, P6 = pp["P3"], pp["P4"], pp["P5"], pp["P6"]

                        # x+1-shifted input slabs straight from DRAM
                        z1t = xpool.tile([P, 17, NK], BF16, tag="z1t")
                        nc.sync.dma_start(z1t[:], z_d[b, x0 + 1:x0 + P + 1, o1:o2 + 1])
                        by1t = xpool.tile([P, 17, NK], BF16, tag="by1t")
                        nc.sync.dma_start(by1t[:], t_d[b, 1, x0 + 1:x0 + P + 1, o1:o2 + 1])

                        # --- base fields ---
                        dz = wpool.tile([P, 18, NK - 1], BF16, tag="dz")
                        nc.vector.tensor_tensor(dz[:], zt[:, y0:y0 + 18, 1:64],
                                                zt[:, y0:y0 + 18, 0:63], OP.subtract)
                        nc.scalar.activation(adz17[:, :, 0:63], dz[:, 1:18, :], AF.Abs)
                        nc.scalar.activation(dz2[:, :, 0:63], dz[:], AF.Square)

                        # x+1 shifts of computed fields (sbuf->sbuf DMA;
                        # partition 127 stays zero from the init memset)
                        nc.sync.dma_start(adzx[0:P - 1], adz17[1:P])

                        # --- std partial: sum_y dz2 over owned rows ---
                        tr2 = mpool.tile([P, NK - 1], F32, tag="tr2")
                        nc.vector.reduce_sum(
                            tr2[:], dz2[:, 1:17, 0:63].rearrange("p y k -> p k y"),
                            axis=AX.X)
                        nc.vector.tensor_tensor(sg2[:], sg2[:], tr2[:], OP.add)

                        # --- smooth: lap in psum via 5 matmuls ---
                        lap_ps = psL.tile([P, 16, NK], F32, tag="lap")
                        mm(lap_ps, 2, dz2[:, 1:17, 1:65], P, True, False)
                        mm(lap_ps, 3, dz2[:, 0:16, 1:65], P, False, False)
                        mm(lap_ps, 3, dz2[:, 2:18, 1:65], P, False, False)
                        mm(lap_ps, 3, dz2[:, 1:17, 0:64], P, False, False)
                        mm(lap_ps, 3, dz2[:, 1:17, 2:66], P, False, True)

                        lscr = wpool.tile([P, 16, NK - 3], BF16, tag="lscr")
                        lcol = mpool.tile([P, 1], F32, tag="lcol")
                        lecol = mpool.tile([P, 1], F32, tag="lecol")
                        lecol2 = mpool.tile([P, 1], F32, tag="lecol2")
                        if g == 0:
                            lmain, ledge, lflag = (1, 16), (0, 1), 6
                        else:
                            lmain, ledge, lflag = (0, 15), (15, 16), 7
                        nc.scalar.activation(
                            lscr[:, 0:lmain[1] - lmain[0], :],
                            lap_ps[:, lmain[0]:lmain[1], 0:61], AF.Square,
                            accum_out=lcol[:])
                        nc.scalar.activation(
                            lscr[:, 15:16, :],
                            lap_ps[:, ledge[0]:ledge[1], 0:61], AF.Square,
                            accum_out=lecol[:])
                        nc.vector.tensor_scalar_mul(lecol2[:], lecol[:],
                                                    aux[0:P, lflag:lflag + 1])
                        nc.vector.tensor_tensor(lapc[:], lapc[:], lcol[:], OP.add)
                        nc.vector.tensor_tensor(lapc[:], lapc[:], lecol2[:], OP.add)

                        # --- bx family ---
                        ybx = wpool.tile([P, 16, NK], BF16, tag="ybx")
                        nc.gpsimd.tensor_tensor(ybx[:], bxt[:, o1:o2, :],
                                                bxt[:, o1 + 1:o2 + 1, :], OP.add)
                        nc.sync.dma_start(ybxx[0:P - 1, :, :], ybx[1:P, :, :])
                        nc.vector.tensor_tensor(Qbx[:, :, 0:63], ybx[:, :, 0:63],
                                                ybx[:, :, 1:64], OP.add)
                        Padz = wpool.tile([P, 16, NK - 1], BF16, tag="Padz")
                        nc.vector.tensor_tensor(Padz[:], adz17[0:P, 0:16, 0:63],
                                                adz17[0:P, 1:17, 0:63], OP.add)
                        nc.vector.tensor_tensor(P1[:, :, 0:63], Qbx[:, :, 0:63],
                                                Padz[:], OP.mult)
                        g1 = wpool.tile([P, 16, NK], BF16, tag="g1")
                        nc.vector.tensor_tensor(g1[:], bxt[:, o1:o2, :], ybxx[0:P], OP.add)
                        g2 = wpool.tile([P, 16, NK], BF16, tag="g2")
                        nc.vector.tensor_tensor(g2[:], bxt[:, o1 + 1:o2 + 1, :],
                                                ybxx[0:P], OP.add)
                        s8x_ps = psA.tile([P, 16, NK], F32, tag="pb")
                        mm(s8x_ps, 0, Qbx[:], P, True, True)
                        d1 = wpool.tile([P, 16, NK], BF16, tag="d1")
                        nc.scalar.activation(d1[:], s8x_ps[:], AF.Square, scale=0.125)

                        # --- by family ---
                        xby = wpool.tile([P, 17, NK], BF16, tag="xby")
                        nc.gpsimd.tensor_tensor(xby[:], byt[:, o1:o2 + 1, :],
                                                by1t[:], OP.add)
                        nc.vector.tensor_tensor(KRby[:, :, 0:63], xby[:, :, 0:63],
                                                xby[:, :, 1:64], OP.add)
                        s8y_ps = psA.tile([P, 16, NK], F32, tag="pb")
                        mm(s8y_ps, 7, KRby[:, 0:16, 0:64], P, True, False)
                        mm(s8y_ps, 7, KRby[:, 1:17, 0:64], P, False, True)
                        d2 = wpool.tile([P, 16, NK], BF16, tag="d2")
                        nc.scalar.activation(d2[:], s8y_ps[:], AF.Square, scale=0.125)
                        Radz = wpool.tile([P, 17, NK - 1], BF16, tag="Radz")
                        nc.vector.tensor_tensor(Radz[:], adz17[0:P, :, 0:63],
                                                adzx[0:P, :, 0:63], OP.add)
                        nc.vector.tensor_tensor(P2[:, :, 0:63], KRby[:, :, 0:63],
                                                Radz[:], OP.mult)
                        g3 = wpool.tile([P, 16, NK], BF16, tag="g3")
                        nc.vector.tensor_tensor(g3[:], xby[:, 1:17, :],
                                                by1t[:, 0:16, :], OP.add)
                        g4 = wpool.tile([P, 16, NK], BF16, tag="g4")
                        nc.gpsimd.tensor_tensor(g4[:], byt[:, o1:o2, :],
                                                xby[:, 1:17, :], OP.add)

                        # --- z family ---
                        dyz = wpool.tile([P, 16, NK], BF16, tag="dyz")
                        nc.gpsimd.tensor_tensor(dyz[:], zt[:, o1:o2, :],
                                                zt[:, o1 + 1:o2 + 1, :], OP.subtract)
                        nc.sync.dma_start(dyzx[0:P - 1, :, :], dyz[1:P, :, :])
                        dzx = wpool.tile([P, 17, NK], BF16, tag="dzx")
                        nc.vector.tensor_tensor(dzx[:], zt[:, o1:o2 + 1, :],
                                                z1t[:], OP.subtract)
                        nc.vector.tensor_tensor(P3[:, :, 0:64], g1[:],
                                                dzx[:, 0:16, :], OP.mult)
                        nc.vector.tensor_tensor(P4[:, :, 0:64], g2[:],
                                                dzx[:, 1:17, :], OP.mult)
                        nc.vector.tensor_tensor(P5[:, :, 0:64], g3[:], dyzx[0:P], OP.mult)
                        nc.vector.tensor_tensor(P6[:, :, 0:64], g4[:], dyz[:], OP.mult)

                        # --- bz family (ybz and s8z fused into PE accumulation) ---
                        Mp_ps = psA.tile([P, 16, NK], F32, tag="pb")
                        mm(Mp_ps, 1, bzt[:, o1:o2, :], P, True, False)
                        mm(Mp_ps, 1, bzt[:, o1 + 1:o2 + 1, :], P, False, True)
                        nc.scalar.activation(Msb[:, :, 0:64], Mp_ps[:], AF.Copy)
                        s8z_ps = psA.tile([P, 16, NK], F32, tag="pb")
                        mm(s8z_ps, 7, Msb[:, :, 0:64], P, True, False)
                        mm(s8z_ps, 7, Msb[:, :, 1:65], P, False, True)
                        d3 = wpool.tile([P, 16, NK], BF16, tag="d3")
                        nc.scalar.activation(d3[:], s8z_ps[:], AF.Square, scale=0.5)

                        # --- den in psum ---
                        den_ps = psA.tile([P, 16, NK], F32, tag="pb")
                        mm(den_ps, 7, d1[:], P, True, False)
                        mm(den_ps, 7, d2[:], P, False, False)
                        mm(den_ps, 7, d3[:], P, False, True)

                        # --- num in psum via 11 matmuls ---
                        num_ps = psN.tile([P, 16, NK], F32, tag="num")
                        mm(num_ps, 4, P1[:], P, True, False)
                        mm(num_ps, 5, P2[:, 1:17, 0:64], P, False, False)
                        mm(num_ps, 6, P2[:, 0:16, 0:64], P, False, False)
                        mm(num_ps, 7, Msb[:, :, 1:65], P, False, False)
                        mm(num_ps, 3, Msb[:, :, 0:64], P, False, False)
                        mm(num_ps, 8, P3[:, :, 1:65], P, False, False)
                        mm(num_ps, 9, P3[:, :, 0:64], P, False, False)
                        mm(num_ps, 8, P4[:, :, 1:65], P, False, False)
                        mm(num_ps, 9, P4[:, :, 0:64], P, False, False)
                        mm(num_ps, 8, P5[:, :, 1:65], P, False, False)
                        mm(num_ps, 9, P5[:, :, 0:64], P, False, False)
                        mm(num_ps, 8, P6[:, :, 1:65], P, False, False)
                        mm(num_ps, 9, P6[:, :, 0:64], P, False, True)

                        q = wpool.tile([P, 16, NK - 1], BF16, tag="q")
                        nc.scalar.activation(q[:], num_ps[:, :, 0:63], AF.Square)
                        rec = wpool.tile([P, 16, NK - 1], F32, tag="rec")
                        nc.vector.reciprocal_approx_fast(rec[:], den_ps[:, :, 0:63])

                        # --- div reduce (main + edge) ---
                        scr = wpool.tile([P, 16, NK - 1], BF16, tag="scr")
                        dcol = mpool.tile([P, 1], F32, tag="dcol")
                        if g == 0:
                            nc.vector.scalar_tensor_tensor(
                                scr[:], q[:], 1.0, rec[:],
                                OP.mult, OP.mult, accum_out=dcol[:])
                            nc.vector.tensor_tensor(divc[:], divc[:], dcol[:], OP.add)
                        else:
                            nc.vector.scalar_tensor_tensor(
                                scr[:, 0:15, :], q[:, 0:15, :], 1.0,
                                rec[:, 0:15, :],
                                OP.mult, OP.mult, accum_out=dcol[:])
                            nc.vector.tensor_tensor(divc[:], divc[:], dcol[:], OP.add)
                            decol = mpool.tile([P, 1], F32, tag="decol")
                            decol2 = mpool.tile([P, 1], F32, tag="decol2")
                            nc.vector.scalar_tensor_tensor(
                                scr[:, 15:16, :], q[:, 15:16, :], 1.0,
                                rec[:, 15:16, :],
                                OP.mult, OP.mult, accum_out=decol[:])
                            nc.vector.tensor_scalar_mul(decol2[:], decol[:],
                                                        aux[0:P, 8:9])
                            nc.vector.tensor_tensor(divc[:], divc[:], decol2[:], OP.add)

                    # --- apply x-ownership masks, accumulate into globals ---
                    ms2 = mpool.tile([P, NK - 1], F32, tag="ms2")
                    nc.vector.tensor_scalar_mul(ms2[:], sg2[:], aux[0:P, ti:ti + 1])
                    nc.vector.tensor_tensor(s2a[b][0:P, :], s2a[b][0:P, :], ms2[:], OP.add)
                    ml = mpool.tile([P, 1], F32, tag="ml")
                    nc.vector.tensor_scalar_mul(ml[:], lapc[:], aux[0:P, 2 + ti:3 + ti])
                    nc.vector.tensor_tensor(lapa[0:P, :], lapa[0:P, :], ml[:], OP.add)
                    md = mpool.tile([P, 1], F32, tag="md")
                    nc.vector.tensor_scalar_mul(md[:], divc[:], aux[0:P, 4 + ti:5 + ti])
                    nc.vector.tensor_tensor(diva[0:P, :], diva[0:P, :], md[:], OP.add)

            for b in range(NB):
                nc.sync.dma_start(s2_d[b], s2a[b][:])
            nc.sync.dma_start(sc_d[0], lapa[:, 0:1])
            nc.sync.dma_start(sc_d[1], diva[:, 0:1])

    nc.compile()
    return nc


def get_nc():
    global _NC_CACHE
    if _NC_CACHE is None:
        _NC_CACHE = _build_nc()
    return _NC_CACHE


def _make_mats():
    I = np.eye(128, dtype=np.float32)
    U = np.eye(128, k=-1, dtype=np.float32)   # out[p] = in[p+1]
    V = np.eye(128, k=1, dtype=np.float32)    # out[p] = in[p-1]
    mats = np.stack([
        I + U,                 # 0: s8x
        0.25 * (I + U),        # 1: Mp
        6 * I - U - V,         # 2: lap center+x
        -I,                    # 3: lap neighbors / num -Msb
        0.125 * DY * (U - I),  # 4: num P1 x-diff
        0.125 * DX * I,        # 5: num +P2_y1
        -0.125 * DX * I,       # 6: num -P2
        I,                     # 7: num +Msb_k1 / den
        (1.0 / 6.0) * I,       # 8: num +Pi_k1
        -(1.0 / 6.0) * I,      # 9: num -Pi
    ]).astype(BF16_NP)
    return np.ascontiguousarray(mats.transpose(1, 0, 2))


def make_in_maps(outputs, targets):
    outputs = np.asarray(outputs, dtype=np.float32)
    targets = np.asarray(targets, dtype=np.float32)
    z = outputs[:, 0]                                         # (2,256,256,64)
    zp = np.pad(z, ((0, 0), (0, 0), (1, 1), (0, 0))).astype(BF16_NP)
    tp = np.pad(targets, ((0, 0), (0, 0), (0, 0), (1, 1), (0, 0))).astype(BF16_NP)

    mats = _make_mats()

    def xmask(ranges):
        m = np.zeros((2, 128), np.float32)
        for i, (a, bnd) in enumerate(ranges):
            m[i, a:bnd] = 1.0
        return m

    m_std = xmask([(0, 126), (0, 126)])
    m_lap = xmask([(1, 127), (1, 127)])
    m_div = xmask([(0, 127), (1, 127)])

    in_maps = []
    for c in range(NCORES):
        aux = np.zeros((128, 16), np.float32)
        aux[:, 0:2] = m_std.T
        aux[:, 2:4] = m_lap.T
        aux[:, 4:6] = m_div.T
        aux[:, 6] = 0.0 if c == 0 else 1.0             # lap y-low edge valid?
        aux[:, 7] = 0.0 if c == NCORES - 1 else 1.0    # lap y-high edge
        aux[:, 8] = 0.0 if c == NCORES - 1 else 1.0    # div y-high edge
        zslab = np.ascontiguousarray(zp[:, :, 32 * c:32 * c + YSLAB, :])
        tslab = np.ascontiguousarray(tp[:, :, :, 32 * c:32 * c + YSLAB, :])
        in_maps.append({"zslab": zslab, "tslab": tslab,
                        "mats": mats, "aux": aux})
    return in_maps


def _host_strip(z, targets):
    """f64 contributions of the x>=252 strip (std S2, lap^2 sum, div sum)
    plus the telescoped S1 sums over the FULL grid."""
    z64 = z.astype(np.float64)
    # S1 telescopes: sum_xy dz[..., kk] = Sz[kk+1] - Sz[kk]
    Sz = z64.sum(axis=(1, 2))                      # (NB, NK)
    S1 = Sz[:, 1:] - Sz[:, :-1]                    # (NB, NK-1)

    zs = z64[:, 252:256]                           # (NB, 4, 256, 64)
    dzs = zs[..., 1:] - zs[..., :-1]
    dz2s = dzs ** 2
    S2h = dz2s.sum(axis=(1, 2))                    # (NB, NK-1)

    lap = (6 * dz2s[:, 1:3, 1:255, 1:62]
           - dz2s[:, 0:2, 1:255, 1:62] - dz2s[:, 2:4, 1:255, 1:62]
           - dz2s[:, 1:3, 0:254, 1:62] - dz2s[:, 1:3, 2:256, 1:62]
           - dz2s[:, 1:3, 1:255, 0:61] - dz2s[:, 1:3, 1:255, 2:63])
    lap2h = float((lap ** 2).sum())

    # div on strip cells cx in {253, 254} -> cell idx 1:3 of the 4-plane slab
    bxs = targets[:, 0, 252:256].astype(np.float64)
    bys = targets[:, 1, 252:256].astype(np.float64)
    bzs = targets[:, 2, 252:256].astype(np.float64)

    def corner(a, i, j, k):
        sl = lambda bnd: slice(1, None) if bnd else slice(None, -1)
        return a[:, sl(i), sl(j), sl(k)]

    corners = lambda a: {(i, j, k): corner(a, i, j, k)
                         for i in (0, 1) for j in (0, 1) for k in (0, 1)}
    BX, BY, BZ, Z = corners(bxs), corners(bys), corners(bzs), corners(zs)
    az = lambda a, bb: np.abs(Z[a] - Z[bb])
    num = (0.25 * (BX[1,0,0] + BX[1,1,0] + BX[1,0,1] + BX[1,1,1]) * DY * 0.5 * (az((1,0,1),(1,0,0)) + az((1,1,1),(1,1,0)))
           - 0.25 * (BX[0,0,0] + BX[0,1,0] + BX[0,0,1] + BX[0,1,1]) * DY * 0.5 * (az((0,0,1),(0,0,0)) + az((0,1,1),(0,1,0)))
           + 0.25 * (BY[0,1,0] + BY[1,1,0] + BY[0,1,1] + BY[1,1,1]) * DX * 0.5 * (az((0,1,1),(0,1,0)) + az((1,1,1),(1,1,0)))
           - 0.25 * (BY[0,0,0] + BY[1,0,0] + BY[0,0,1] + BY[1,0,1]) * DX * 0.5 * (az((0,0,1),(0,0,0)) + az((1,0,1),(1,0,0)))
           + 0.25 * (BZ[0,0,1] + BZ[0,1,1] + BZ[1,0,1] + BZ[1,1,1]) * DX * DY
           - 0.25 * (BZ[0,0,0] + BZ[0,1,0] + BZ[1,0,0] + BZ[1,1,0]) * DX * DY
           + (BX[0,0,1] + BX[1,0,1] + BX[1,1,1]) * DY * (Z[0,0,1] - Z[1,0,1]) / 6
           + (BX[0,1,1] + BX[1,1,1] + BX[1,0,1]) * DY * (Z[0,1,1] - Z[1,1,1]) / 6
           + (BY[1,0,1] + BY[1,1,1] + BY[0,1,1]) * DX * (Z[1,0,1] - Z[1,1,1]) / 6
           + (BY[0,0,1] + BY[0,1,1] + BY[1,1,1]) * DX * (Z[0,0,1] - Z[0,1,1]) / 6
           - (BX[0,0,0] + BX[1,0,0] + BX[1,1,0]) * DY * (Z[0,0,0] - Z[1,0,0]) / 6
           - (BX[0,1,0] + BX[1,1,0] + BX[1,0,0]) * DY * (Z[0,1,0] - Z[1,1,0]) / 6
           - (BY[1,0,0] + BY[1,1,0] + BY[0,1,0]) * DX * (Z[1,0,0] - Z[1,1,0]) / 6
           - (BY[0,0,0] + BY[0,1,0] + BY[1,1,0]) * DX * (Z[0,0,0] - Z[0,1,0]) / 6)
    sum8 = lambda C: sum(C.values())
    den = ((sum8(BX) * 0.125) ** 2 + (sum8(BY) * 0.125) ** 2
           + (sum8(BZ) * 0.125) ** 2 + EPS)
    divh = float((num[:, 1:3] ** 2 / den[:, 1:3]).sum())
    return S1, S2h, lap2h, divh


def combine(results, outputs, targets):
    outputs = np.asarray(outputs, dtype=np.float32)
    targets = np.asarray(targets, dtype=np.float32)
    z = outputs[:, 0]
    S1, S2, lap2, divs = _host_strip(z, targets)
    S2 = S2.copy()
    for r in results:
        S2 += r["o_s2"].astype(np.float64).sum(axis=1)
        lap2 += float(r["o_sc"][0].astype(np.float64).sum())
        divs += float(r["o_sc"][1].astype(np.float64).sum())
    N = NX * NY
    var = (S2 - S1 * S1 / N) / (N - 1)
    loss_std = np.mean(np.sqrt(np.maximum(var, 0.0)))
    loss_smooth = lap2 / (NB * 254 * 254 * 61)
    loss_div = divs / (NB * 255 * 255 * 63)
    return (np.float32(loss_div * W_DIV),
            np.float32(loss_smooth * W_SMOOTH + loss_std * W_STD))


def kernel(outputs, targets):
    import os
    # NTFF tracing needs antenv.axon_hooks, absent in this container; make
    # sure a stray BASS_TRACE in the environment can't break the run.
    os.environ["BASS_NEVER_TRACE"] = "1"
    from concourse.bass_utils import run_bass_kernel_spmd

    nc = get_nc()
    in_maps = make_in_maps(outputs, targets)
    res = run_bass_kernel_spmd(nc, in_maps, list(range(NCORES)))
    return combine(res.results, outputs, targets)
